# revision 1
# baseline (speedup 1.0000x reference)
import numpy as np

# GatedDeltaNet, hardcoded problem dims (nn_GatedDeltaNet_50766513438887)
B, L, HID, NH, DK, DV, K = 4, 2048, 2048, 6, 256, 256, 4
KD, VD = NH * DK, NH * DV
EPS = 1e-5
NCORE = 8
HPC = (B * NH) // NCORE  # (batch,head) pairs per core = 3
SCALE = DK ** -0.5

_jit_fn = None


def _build_jax():
    import jax
    import jax.numpy as jnp
    from jax.sharding import Mesh, PartitionSpec as P
    from jax.experimental.shard_map import shard_map
    from functools import partial

    devs = np.array(jax.devices()[:NCORE])
    mesh = Mesh(devs, ("c",))

    def _conv_silu(y, w):
        # y: (L, C) time-major, w: (C, K) depthwise causal conv + SiLU
        yp = jnp.pad(y, ((K - 1, 0), (0, 0)))
        acc = yp[0:L] * w[:, 0]
        for j in range(1, K):
            acc = acc + yp[j:j + L] * w[:, j]
        return acc * jax.nn.sigmoid(acc)

    def _l2n(v):
        return v / jnp.maximum(jnp.sqrt(jnp.sum(v * v, axis=-1, keepdims=True)), 1e-12)

    def per_core(x, Wq, Wk, Wv, Wb, Wa, A_log, dt_bias, cq, ck, cv, Wg, norm_w, Wo):
        idx = jax.lax.axis_index("c")
        b = idx // 2            # batch handled by this core
        h0 = 3 * (idx % 2)      # first of 3 heads handled by this core
        HC = HPC                # heads per core
        xb = jax.lax.dynamic_slice(x, (b, 0, 0), (1, L, HID))[0]          # (L, HID)

        def hslice(W, d):       # slice d columns-per-head starting at head h0
            return jax.lax.dynamic_slice(W, (0, h0 * d), (HID, HC * d))

        def cslice(cw, d):      # conv weights for our heads
            return jax.lax.dynamic_slice(cw, (h0 * d, 0), (HC * d, K))

        q = _conv_silu(xb @ hslice(Wq, DK), cslice(cq, DK)).reshape(L, HC, DK)
        k = _conv_silu(xb @ hslice(Wk, DK), cslice(ck, DK)).reshape(L, HC, DK)
        v = _conv_silu(xb @ hslice(Wv, DV), cslice(cv, DV)).reshape(L, HC, DV)
        beta = jax.nn.sigmoid(xb @ jax.lax.dynamic_slice(Wb, (0, h0), (HID, HC)))  # (L, HC)
        A = jax.lax.dynamic_slice(A_log, (h0,), (HC,))
        db = jax.lax.dynamic_slice(dt_bias, (h0,), (HC,))
        g = -jnp.exp(A) * jax.nn.softplus(xb @ jax.lax.dynamic_slice(Wa, (0, h0), (HID, HC)) + db)
        gate = (xb @ hslice(Wg, DV)).reshape(L, HC, DV)

        def step(state, inp):
            q_t, k_t, v_t, g_t, b_t = inp
            kn = _l2n(k_t)
            qn = _l2n(q_t) * SCALE
            state = state * jnp.exp(g_t)[:, None, None]
            corr = jnp.einsum("hk,hkv->hv", kn, state)
            u = (v_t - corr) * b_t[:, None]
            state = state + jnp.einsum("hk,hv->hkv", kn, u)
            o_t = jnp.einsum("hk,hkv->hv", qn, state)
            return state, o_t

        s0 = jnp.zeros((HC, DK, DV), jnp.float32)
        fs, o = jax.lax.scan(step, s0, (q, k, v, g, beta))               # o: (L, HC, DV)

        o_n = o * jax.lax.rsqrt(jnp.mean(o * o, axis=-1, keepdims=True) + EPS) * norm_w
        o_n = o_n * (gate * jax.nn.sigmoid(gate))
        Wo_h = jax.lax.dynamic_slice(Wo, (h0 * DV, 0), (HC * DV, HID))
        part = o_n.reshape(L, HC * DV) @ Wo_h                            # (L, HID) partial
        return part[None], fs[None]

    rep = P()
    fn = shard_map(
        per_core, mesh=mesh,
        in_specs=(rep,) * 14,
        out_specs=(P("c"), P("c")),
        check_rep=False,
    )

    @jax.jit
    def full(*args):
        parts, fs = fn(*args)                    # (8, L, HID), (8, HPC, DK, DV)
        out = parts.reshape(B, 2, L, HID).sum(1)
        final_state = fs.reshape(B, NH, DK, DV)
        return out, final_state

    return full


def _kernel_numpy(x, Wq, Wk, Wv, Wb, Wa, A_log, dt_bias, conv_q, conv_k, conv_v,
                  Wg, norm_w, Wo):
    def conv_silu(y, w):
        yp = np.pad(y, ((0, 0), (K - 1, 0), (0, 0)))
        acc = sum(yp[:, j:j + L] * w[:, j] for j in range(K))
        return acc / (1.0 + np.exp(-acc)) * 1.0 if False else acc * (1.0 / (1.0 + np.exp(-acc)))

    def l2n(v):
        return v / np.maximum(np.sqrt(np.sum(v * v, axis=-1, keepdims=True)), 1e-12)

    q = conv_silu(x @ Wq, conv_q).reshape(B, L, NH, DK)
    k = conv_silu(x @ Wk, conv_k).reshape(B, L, NH, DK)
    v = conv_silu(x @ Wv, conv_v).reshape(B, L, NH, DV)
    beta = 1.0 / (1.0 + np.exp(-(x @ Wb)))
    g = -np.exp(A_log) * np.logaddexp(0.0, x @ Wa + dt_bias)
    gate = (x @ Wg).reshape(B, L, NH, DV)

    state = np.zeros((B, NH, DK, DV), np.float32)
    o = np.empty((B, L, NH, DV), np.float32)
    for t in range(L):
        kn = l2n(k[:, t])
        qn = l2n(q[:, t]) * SCALE
        state *= np.exp(g[:, t])[..., None, None]
        corr = np.einsum("bhk,bhkv->bhv", kn, state)
        u = (v[:, t] - corr) * beta[:, t][..., None]
        state += np.einsum("bhk,bhv->bhkv", kn, u)
        o[:, t] = np.einsum("bhk,bhkv->bhv", qn, state)

    o_n = o / np.sqrt(np.mean(o * o, axis=-1, keepdims=True) + EPS) * norm_w
    o_n = o_n * (gate / (1.0 + np.exp(-gate)))
    out = o_n.reshape(B, L, VD) @ Wo
    return out.astype(np.float32), state.astype(np.float32)


def kernel(**inputs):
    order = ["x", "Wq", "Wk", "Wv", "Wb", "Wa", "A_log", "dt_bias",
             "conv_q", "conv_k", "conv_v", "Wg", "norm_w", "Wo"]
    args = [np.asarray(inputs[n], np.float32) for n in order]
    global _jit_fn
    try:
        import jax
        if _jit_fn is None:
            _jit_fn = _build_jax()
        out, fs = _jit_fn(*args)
        return np.asarray(out, np.float32), np.asarray(fs, np.float32)
    except Exception:
        return _kernel_numpy(*args)
